# revision 26
# baseline (speedup 1.0000x reference)
"""Trainium2 Bass kernel for nn_EuclideanGATLayer (GAT layer, N=8192).

Math (per reference):
    Wh = h @ W                     [N, F]
    s  = Wh @ a[:F]   (e_src)      [N]
    d  = Wh @ a[F:]   (e_dst)      [N]
    e_ij = leaky_relu(s_i + d_j, 0.01)
    attn = softmax(where(adj>0, e, -9e15), axis=1)
    out  = elu(attn @ Wh)

Key reformulation (one transcendental pass over the N^2 matrix total):
    leaky_relu(x) = 0.01*x + 0.99*relu(x)
    exp(lrelu(s_i+d_j)) = exp(0.01*s_i) * exp(0.01*d_j) * exp(0.99*relu(s_i+d_j))
    The row factor exp(0.01*s_i) cancels in the softmax ratio. The column
    factor Q_j = exp(0.01*d_j) is folded into G = [Q*Wh | Q].
    num_i|den_i = Z @ G with Z_ij = adj_ij * exp(0.99*relu(s_i+d_j)),
    out = elu(num/den).

Masking is fused into the HBM load: adjT is host-staged as bf16 {0, BIG}
(a lossless packing of the 0/1 int32 input at half the bytes); the SWDGE
compute-DMA widens it to fp32 and *adds* it onto relu(x), and the Exp bias
of -0.99*BIG restores neighbors while non-neighbors underflow to exactly 0
(exp(0.99*relu - 198) = 0 in fp32 since relu <= ~70).

Distribution: row-shard N across 8 cores (1024 rows each). adj is staged
host-side in transposed fp32 layout (same byte count as the int32 input) so
the matmul contraction dim (j) lands in the partition dimension with fully
contiguous DMA; matmuls run in float32r (full-rate fp32 data path).
"""
import sys

sys.path.insert(0, "/opt/trn_rl_repo")

import numpy as np
from contextlib import ExitStack

import ml_dtypes
import concourse.bass as bass
import concourse.bacc as bacc
import concourse.tile as tile
from concourse import mybir
from concourse import bass_utils
from concourse.masks import make_identity

N_FULL = 8192
IN_DIM = 128
F = 64
NCORES = 8
MASK_BIG = 200.0  # adj staged as {0, BIG}; exp(0.99*relu - 0.99*BIG) underflows to 0

f32 = mybir.dt.float32
f32r = mybir.dt.float32r
bf16 = mybir.dt.bfloat16
AF = mybir.ActivationFunctionType
OP = mybir.AluOpType


def gat_body(tc, out, adjT, hT, hTown, W, a, n, rows, repeats=1):
    """Emit the GAT kernel into TileContext tc.

    out:   [rows, F]      DRAM out (this core's row block)
    adjT:  [n, rows]      DRAM in, bf16 {0, MASK_BIG}, transposed adj slice
    hT:    [IN_DIM, n]    DRAM in, h transposed (full)
    hTown: [IN_DIM, rows] DRAM in, this core's columns of hT
    W:     [IN_DIM, F]    DRAM in
    a:     [2*F]          DRAM in
    """
    nc = tc.nc
    nchunks = n // 128
    CPB = 4  # j-chunks per batch
    GRP = 8  # Wh chunks per group tile (fine-grained deps for early start)
    assert nchunks % CPB == 0 and nchunks % GRP == 0
    nbatches = nchunks // CPB
    ngroups = nchunks // GRP
    assert rows % 256 == 0
    NIT = rows // 256  # number of [65,256] psum accumulators
    NBLK = rows // 128  # output row blocks

    with ExitStack() as ctx:
        const = ctx.enter_context(tc.tile_pool(name="const", bufs=1))

        # ---- small inputs first (hTown before hT: the s-chain needs it) ----
        hTownsb = const.tile([128, rows], f32)
        nc.sync.dma_start(out=hTownsb, in_=hTown)
        Wsb = const.tile([128, F], f32)
        nc.sync.dma_start(out=Wsb, in_=W)
        a1b = const.tile([128, F], f32)
        a2b = const.tile([128, F], f32)
        nc.sync.dma_start(
            out=a1b, in_=bass.AP(tensor=a.tensor, offset=a.offset, ap=[[0, 128], [1, F]])
        )
        nc.sync.dma_start(
            out=a2b,
            in_=bass.AP(tensor=a.tensor, offset=a.offset + F, ap=[[0, 128], [1, F]]),
        )
        # hT in group tiles so Wh group 0 starts after ~1 chunk of DMA
        hTg = [const.tile([128, GRP * 128], f32, name=f"hTg{g}") for g in range(ngroups)]
        for g in range(ngroups):
            nc.sync.dma_start(
                out=hTg[g], in_=hT[:, g * GRP * 128 : (g + 1) * GRP * 128]
            )

        tmp1 = const.tile([128, F], f32)
        nc.vector.tensor_tensor(out=tmp1, in0=Wsb, in1=a1b, op=OP.mult)
        Wa1 = const.tile([128, 1], f32)
        nc.vector.tensor_reduce(out=Wa1, in_=tmp1, axis=mybir.AxisListType.X, op=OP.add)
        tmp2 = const.tile([128, F], f32)
        nc.vector.tensor_tensor(out=tmp2, in0=Wsb, in1=a2b, op=OP.mult)
        Wa2 = const.tile([128, 1], f32)
        nc.vector.tensor_reduce(out=Wa2, in_=tmp2, axis=mybir.AxisListType.X, op=OP.add)

        # W2 = [W | Wa2 | Wa1]  (cols 0:64 = W, 64 = Wa2 (-> d), 65 = Wa1 (-> s))
        W2 = const.tile([128, F + 2], f32)
        nc.vector.tensor_copy(W2[:, :F], Wsb)
        nc.vector.tensor_copy(W2[:, F : F + 1], Wa2)
        nc.vector.tensor_copy(W2[:, F + 1 : F + 2], Wa1)

        ident = const.tile([128, 128], f32)
        make_identity(nc, ident)
        ebias = const.tile([128, 1], f32)
        nc.vector.memset(ebias, -0.99 * MASK_BIG)
        ones1 = const.tile([1, 128], f32)
        nc.vector.memset(ones1, 1.0)

        s_rowsb = const.tile([1, rows], f32)
        s_bcast = const.tile([128, rows], f32)
        Whg = [
            const.tile([128, GRP, F + 2], f32, name=f"Whg{g}") for g in range(ngroups)
        ]

        with ExitStack() as sctx:
            spool = sctx.enter_context(
                tc.tile_pool(name="setup_ps", bufs=2, space="PSUM")
            )
            # s (e_src for own rows) as a row vector: s = Wa1.T @ hTown
            s_ps = spool.tile([1, rows], f32, tag="s", bufs=1)
            for j0 in range(0, rows, 512):
                nc.tensor.matmul(
                    s_ps[:, j0 : j0 + 512],
                    Wa1,
                    hTownsb[:, j0 : j0 + 512],
                    start=True,
                    stop=True,
                )
            nc.vector.tensor_copy(s_rowsb, s_ps)
            # broadcast s across partitions: ones[128].T @ s_row
            for j0 in range(0, rows, 512):
                sb_ps = spool.tile([128, 512], f32, tag="sb")
                nc.tensor.matmul(
                    sb_ps, ones1, s_rowsb[:, j0 : j0 + 512], start=True, stop=True
                )
                nc.vector.tensor_copy(s_bcast[:, j0 : j0 + 512], sb_ps)

            # Wh'' = hT.T @ W2 (grouped); Q/G built per group as soon as
            # that group's Wh lands so early batches' matmuls aren't gated
            # on the full setup.
            Qg = [const.tile([128, GRP], f32, name=f"Qg{g}") for g in range(ngroups)]
            Gg = [
                const.tile([128, GRP, F + 1], f32r, name=f"Gg{g}")
                for g in range(ngroups)
            ]
            for g in range(ngroups):
                for cc in range(GRP):
                    wh_ps = spool.tile([128, F + 2], f32, tag="wh")
                    nc.tensor.matmul(
                        wh_ps,
                        hTg[g][:, cc * 128 : (cc + 1) * 128],
                        W2,
                        start=True,
                        stop=True,
                    )
                    nc.vector.tensor_copy(Whg[g][:, cc, :], wh_ps)
                nc.scalar.activation(Qg[g], Whg[g][:, :, F], AF.Exp, scale=0.01)
                qb = bass.AP(
                    tensor=Qg[g].tensor,
                    offset=Qg[g].offset,
                    ap=[Qg[g].ap[0], Qg[g].ap[1], [0, F]],
                )
                nc.vector.tensor_tensor(
                    out=Gg[g][:, :, :F], in0=Whg[g][:, :, :F], in1=qb, op=OP.mult
                )
                nc.vector.tensor_copy(Gg[g][:, :, F], Qg[g])

        # ---- main stream over j ----
        rpool = ctx.enter_context(tc.tile_pool(name="rpool", bufs=4))
        epool = ctx.enter_context(tc.tile_pool(name="epool", bufs=3))
        otpool = ctx.enter_context(tc.tile_pool(name="ot_ps", bufs=1, space="PSUM"))
        ot = [
            otpool.tile([F + 1, 256], f32, tag=f"ot{t}", name=f"ot{t}")
            for t in range(NIT)
        ]

        # small ramp-in batches cut the serial preamble (first adj DMA needs
        # only one ts); small ramp-out batches shorten the post-stream tail.
        plan = [1, 1, 2] + [CPB] * ((nchunks - 8) // CPB) + [2, 2]
        assert sum(plan) == nchunks
        for rep in range(repeats):
          c0 = 0
          for cpb in plan:
            rt = rpool.tile([128, CPB, rows], f32, tag="rt")
            for k in range(cpb):
                c = c0 + k
                nc.vector.tensor_scalar(
                    out=rt[:, k, :],
                    in0=s_bcast,
                    scalar1=Whg[c // GRP][:, c % GRP, F : F + 1],
                    scalar2=0.0,
                    op0=OP.add,
                    op1=OP.max,
                )
            adj_in = adjT[c0 * 128 : (c0 + cpb) * 128, :].rearrange(
                "(c p) i -> p c i", p=128
            )
            nc.gpsimd.dma_start(out=rt[:, :cpb, :], in_=adj_in, accum_op=OP.add)
            et = epool.tile([128, CPB, rows], f32r, tag="et")
            nc.scalar.activation(
                et[:, :cpb, :], rt[:, :cpb, :], AF.Exp, scale=0.99, bias=ebias
            )
            for k in range(cpb):
                c = c0 + k
                for t in range(NIT):
                    nc.tensor.matmul(
                        ot[t],
                        Gg[c // GRP][:, c % GRP, :],
                        et[:, k, t * 256 : (t + 1) * 256],
                        start=(rep == 0 and c == 0),
                        stop=(rep == repeats - 1 and c == nchunks - 1),
                    )
            c0 += cpb

        # ---- epilogue (batched): transpose all blocks, then wide ops ----
        small = ctx.enter_context(tc.tile_pool(name="small", bufs=2))
        tpps = ctx.enter_context(tc.tile_pool(name="tp_ps", bufs=2, space="PSUM"))
        hpall = const.tile([128, NBLK, F + 1], f32)
        for half in range(2):
            tp4 = tpps.tile([128, NBLK // 2, F + 1], f32, tag="tp")
            for q in range(NBLK // 2):
                blk = half * (NBLK // 2) + q
                t, h = divmod(blk, 2)
                if h == 0:
                    otsb = small.tile([F + 1, 256], f32, tag="otsb")
                    nc.vector.tensor_copy(otsb, ot[t])
                nc.tensor.transpose(
                    tp4[:, q, :], otsb[:, h * 128 : (h + 1) * 128], ident[: F + 1, : F + 1]
                )
            nc.vector.tensor_copy(hpall[:, half * (NBLK // 2) : (half + 1) * (NBLK // 2), :], tp4)
        dens = const.tile([128, NBLK], f32)
        nc.vector.reciprocal(dens, hpall[:, :, F])
        db = bass.AP(
            tensor=dens.tensor, offset=dens.offset, ap=[dens.ap[0], dens.ap[1], [0, F]]
        )
        hpre = const.tile([128, NBLK, F], f32)
        nc.vector.tensor_tensor(out=hpre, in0=hpall[:, :, :F], in1=db, op=OP.mult)
        # elu(x) = relu(x) - 1 + exp(min(x, 0))
        emin = const.tile([128, NBLK, F], f32)
        nc.vector.tensor_scalar(
            out=emin, in0=hpre, scalar1=0.0, scalar2=None, op0=OP.min
        )
        eexp = const.tile([128, NBLK, F], f32)
        nc.scalar.activation(eexp, emin, AF.Exp)
        relu1 = const.tile([128, NBLK, F], f32)
        nc.vector.tensor_scalar(
            out=relu1, in0=hpre, scalar1=0.0, scalar2=-1.0, op0=OP.max, op1=OP.add
        )
        otf = const.tile([128, NBLK, F], f32)
        nc.vector.tensor_tensor(out=otf, in0=relu1, in1=eexp, op=OP.add)
        nc.sync.dma_start(
            out=out.rearrange("(b p) f -> p b f", p=128), in_=otf
        )


def build_module(n, rows, repeats=1):
    nc = bacc.Bacc(
        "TRN2",
        target_bir_lowering=False,
        debug=False,
        enable_asserts=True,
        num_devices=NCORES,
    )
    adjT = nc.dram_tensor("adjT", [n, rows], bf16, kind="ExternalInput").ap()
    hT = nc.dram_tensor("hT", [IN_DIM, n], f32, kind="ExternalInput").ap()
    hTown = nc.dram_tensor("hTown", [IN_DIM, rows], f32, kind="ExternalInput").ap()
    W = nc.dram_tensor("W", [IN_DIM, F], f32, kind="ExternalInput").ap()
    a = nc.dram_tensor("a", [2 * F], f32, kind="ExternalInput").ap()
    out = nc.dram_tensor("out", [rows, F], f32, kind="ExternalOutput").ap()
    with tile.TileContext(nc) as tc:
        gat_body(tc, out, adjT, hT, hTown, W, a, n, rows, repeats=repeats)
    nc.compile()
    return nc


def make_in_maps(h, adj, W, a, n, rows, ncores):
    hT = np.ascontiguousarray(h.T).astype(np.float32)
    adjTf = np.ascontiguousarray(adj.T).astype(ml_dtypes.bfloat16) * ml_dtypes.bfloat16(MASK_BIG)
    in_maps = []
    for c in range(ncores):
        sl = slice(c * rows, (c + 1) * rows)
        in_maps.append(
            {
                "adjT": np.ascontiguousarray(adjTf[:, sl]),
                "hT": hT,
                "hTown": np.ascontiguousarray(hT[:, sl]),
                "W": np.asarray(W, dtype=np.float32),
                "a": np.asarray(a, dtype=np.float32),
            }
        )
    return in_maps


_nc_cache = {}


def get_module(n=N_FULL, rows=N_FULL // NCORES, repeats=1):
    key = (n, rows, repeats)
    if key not in _nc_cache:
        _nc_cache[key] = build_module(n, rows, repeats)
    return _nc_cache[key]


def kernel(h, adj, W, a, trace=False, trace_kwargs=None):
    h = np.asarray(h, dtype=np.float32)
    adj = np.asarray(adj)
    W = np.asarray(W, dtype=np.float32)
    a = np.asarray(a, dtype=np.float32)
    n = h.shape[0]
    rows = n // NCORES
    nc = get_module(n, rows)
    in_maps = make_in_maps(h, adj, W, a, n, rows, NCORES)
    res = bass_utils.run_bass_kernel_spmd(
        nc,
        in_maps,
        core_ids=list(range(NCORES)),
        trace=trace,
        **(trace_kwargs or {}),
    )
    out = np.concatenate([res.results[c]["out"] for c in range(NCORES)], axis=0)
    kernel.last_results = res
    return out


if __name__ == "__main__":
    rng = np.random.default_rng(0)
    h = rng.standard_normal((N_FULL, IN_DIM), dtype=np.float32)
    adj = (rng.random((N_FULL, N_FULL)) < 0.5).astype(np.int32)
    W = (rng.standard_normal((IN_DIM, F), dtype=np.float32) / np.sqrt(IN_DIM)).astype(
        np.float32
    )
    a = rng.standard_normal(2 * F, dtype=np.float32)
    out = kernel(h, adj, W, a)
    print("out", out.shape, out.dtype, np.abs(out).mean())
